# revision 23
# baseline (speedup 1.0000x reference)
"""Trainium2 Bass kernel for nn_CPA_CCA_block (channel attention + spatial attention + fusion).

Batch-sharded: 8 samples over 8 NeuronCores, replicated weights, zero collectives.

Schedule (single pass, engines overlapped):
  - x streamed in 512-px chunks; per chunk: stats rows (F/avg/bm), channel max
    (Pool all-reduce), PE transposes of x + exact S logits accumulation, and
    the spatial conv for trailing 480-px windows — all pipelined under the DMA.
  - spatial softmax + channel softmax + (w_h1 w_beta S^T w_f)-fold into the
    x-weights right after the load drains.
  - Esp production software-pipelined (PE) with the D-lrelu split Act/DVE and
    PSUM evacuation on gpsimd.
  - fusion streamed in 480-px chunks; Hh/M kept bf16 (1 cycle/row moving).
Cost-model notes: matmul cost = out free size x cycles/row; transposes use a
bf16 identity (1 c/r); f32r moving needs free>=256 for 1 c/r.
"""
import sys
sys.path.insert(0, '/opt/trn_rl_repo')
import numpy as np
from contextlib import ExitStack

import concourse.bacc as bacc
import concourse.tile as tile
from concourse import mybir
from concourse.bass_utils import run_bass_kernel_spmd
from concourse import bass_isa
import ml_dtypes

F32 = mybir.dt.float32
F32R = mybir.dt.float32r
BF16 = mybir.dt.bfloat16
AF = mybir.ActivationFunctionType
ALU = mybir.AluOpType
AX = mybir.AxisListType

B, C, H, W = 8, 256, 96, 96
HW = H * W
K16 = 16
NEG = 0.001
LCH = [(j * 1536, 1536) for j in range(6)]   # load chunks
FCH = [(k * 480, 480) for k in range(19)] + [(9120, 96)]   # conv/fusion chunks


def _round_f32r(a):
    b = np.ascontiguousarray(a, dtype=np.float32).view(np.uint32)
    lsb = (b >> np.uint32(12)) & np.uint32(1)
    r = (b + np.uint32(0x7FF) + lsb) & np.uint32(0xFFFFF000)
    return r.view(np.float32)


def _build_program():
    nc = bacc.Bacc("TRN2", target_bir_lowering=False, debug=False)

    def din(name, shape, dt):
        return nc.dram_tensor(name, shape, dt, kind="ExternalInput").ap()

    X = din("x", [C, HW], F32R)
    Wstr = din("wstr", [C, 18], F32R)
    Wf16 = din("wf16", [K16, C], F32R)
    IDNB = din("idnb", [128, 128], BF16)
    IDNR = din("idnr", [128, 128], F32R)
    Wcol = din("wcol", [14, 16], BF16)
    Wf2d = din("wf2d", [4, C], BF16)
    WF2CB = din("wf2cb", [1, 4], F32)
    Whb = din("whb", [2, 128, C], F32R)
    Whx = din("whx", [2, 128, C], F32R)
    Wm1 = din("wm1", [2, 128, C], F32R)
    Wm2 = din("wm2", [2, 128, C], F32R)
    Whm1 = din("whm1", [2, 128, C], F32R)
    Whm2 = din("whm2", [2, 128, C], F32R)
    Whe = din("whe", [2, 128, C], BF16)

    Y = nc.dram_tensor("y", [C, HW], F32, kind="ExternalOutput").ap()

    def lrelu(out, src):
        nc.scalar.activation(out, src, AF.Prelu, alpha=NEG)

    with tile.TileContext(nc) as tc, ExitStack() as ctx:
        per = ctx.enter_context(tc.tile_pool(name="per", bufs=1))
        x0 = per.tile([128, HW], F32R, tag="x0")
        x1 = per.tile([128, HW], F32R, tag="x1")
        xs = [x0, x1]
        f4r = per.tile([4, HW], BF16, tag="f4r")
        bm_pl = per.tile([96, 96], BF16, tag="bm_pl")

        idnb = per.tile([128, 128], BF16, tag="idnb")
        idnr = per.tile([128, 128], F32R, tag="idnr")
        wstr = [per.tile([128, 18], F32R, tag=f"wstr{t}", name=f"wstr{t}") for t in range(2)]
        wf16 = per.tile([K16, C], F32R, tag="wf16")
        wcol = per.tile([14, 16], BF16, tag="wcol")
        wf2d = per.tile([4, C], BF16, tag="wf2d")
        wf2cb = per.tile([96, 4], F32, tag="wf2cb")
        sspT_bf = per.tile([96, 96], BF16, tag="sspT")
        S_sb = per.tile([K16, C], F32R, tag="S_sb")
        s_pre = per.tile([K16, C], F32, tag="s_pre")
        sT_sb = per.tile([128, 2, K16], F32R, tag="sT_sb")
        Wst2T = per.tile([K16, C], F32R, tag="Wst2T")
        whx_eff = [per.tile([128, C], F32R, tag=f"whx_eff{kt}", name=f"whx_eff{kt}")
                   for kt in range(2)]

        def load_w(name, ap, dt):
            ts = []
            for kt in range(2):
                t = per.tile([128, C], dt, tag=f"{name}{kt}", name=f"{name}{kt}")
                nc.sync.dma_start(t, ap[kt])
                ts.append(t)
            return ts

        # preamble: small weights needed during the load phase
        for t in range(2):
            nc.sync.dma_start(wstr[t], Wstr[t * 128:(t + 1) * 128, :])
        nc.sync.dma_start(idnb, IDNB)
        nc.sync.dma_start(idnr, IDNR)
        nc.sync.dma_start(wf16, Wf16)
        nc.sync.dma_start(wcol, Wcol)
        nc.sync.dma_start(wf2d, Wf2d)
        nc.gpsimd.dma_start(out=wf2cb, in_=WF2CB.to_broadcast((96, 4)))

        # ---------------- load phase: x chunks + stats rows + S logits ----------------
        with tc.tile_pool(name="pAvbm", bufs=1) as pAvbm, \
             tc.tile_pool(name="psS", bufs=1, space="PSUM") as psS:
            # rows: 0=avg(bf16), 1=bm, 2=channel-max
            avbm = pAvbm.tile([3, HW], BF16, tag="avbm")
            sps = psS.tile([K16, C], F32, tag="sps")

            with tc.tile_pool(name="sbL", bufs=1) as sbL, \
                 tc.tile_pool(name="psA", bufs=2, space="PSUM") as psA, \
                 tc.tile_pool(name="psT", bufs=3, space="PSUM") as psT, \
                 tc.tile_pool(name="psStF", bufs=2, space="PSUM") as psStF:
                for j, (n0, n) in enumerate(LCH):
                    for ct in range(2):
                        nc.sync.dma_start(xs[ct][:, n0:n0 + n],
                                          X[ct * 128:(ct + 1) * 128, n0:n0 + n])
                    st_sb = sbL.tile([18, 1536], F32R, tag="st_sb", bufs=2)
                    for half in range(3):
                        a = half * 512
                        psa = psA.tile([18, 512], F32, tag="ps")
                        nc.tensor.matmul(psa, wstr[0], x0[:, n0 + a:n0 + a + 512],
                                         start=True, stop=False)
                        nc.tensor.matmul(psa, wstr[1], x1[:, n0 + a:n0 + a + 512],
                                         start=False, stop=True)
                        if half % 2 == 0:
                            nc.vector.tensor_copy(st_sb[:, a:a + 512], psa)
                        else:
                            nc.scalar.copy(st_sb[:, a:a + 512], psa)
                    # avg/bm rows -> bf16 (gpsimd cast DMA, one per chunk)
                    nc.gpsimd.dma_start(out=avbm[0:2, n0:n0 + n], in_=st_sb.bitcast(F32)[16:18, 0:n])
                    # channel max (Pool) -> row 2
                    mx = sbL.tile([128, 1536], BF16, tag="mx", bufs=2)
                    nc.vector.tensor_tensor(mx[:, 0:n], x0[:, n0:n0 + n].bitcast(F32),
                                            x1[:, n0:n0 + n].bitcast(F32), op=ALU.max)
                    nc.gpsimd.partition_all_reduce(mx[:, 0:n], mx[:, 0:n], channels=128,
                                                   reduce_op=bass_isa.ReduceOp.max)
                    nc.sync.dma_start(avbm[2:3, n0:n0 + n], mx[0:1, 0:n])
                    # F^T chunks for the exact S logits
                    stf_ps = psStF.tile([128, 12, K16], F32R, tag="stf")
                    for s in range(12):
                        nc.tensor.transpose(stf_ps[:, s, :],
                                            st_sb[0:16, s * 128:(s + 1) * 128],
                                            idnr[0:K16, 0:K16])
                    stf_sb = sbL.tile([128, 12, K16], F32R, tag="stf_sb", bufs=2)
                    nc.vector.tensor_copy(stf_sb, stf_ps)
                    # x^T subchunk pairs + S logit accumulation (full-precision path)
                    for sp in range(6):
                        pt = psT.tile([128, 2, 2, 128], F32R, tag="pt")
                        for si in range(2):
                            m0 = n0 + (2 * sp + si) * 128
                            for ct in range(2):
                                nc.tensor.transpose(pt[:, si, ct, :], xs[ct][:, m0:m0 + 128], idnr)
                        xt = sbL.tile([128, 2, 2, 128], F32R, tag="xt", bufs=4)
                        if sp % 3 == 2:
                            nc.vector.tensor_copy(xt, pt)
                        else:
                            nc.scalar.copy(xt, pt)
                        for si in range(2):
                            s = 2 * sp + si
                            nc.tensor.matmul(sps, stf_sb[:, s, :],
                                             xt[:, si, :, :].rearrange("p a b -> p (a b)"),
                                             start=(j == 0 and s == 0),
                                             stop=(j == len(LCH) - 1 and s == 11))

                # remaining big weights (DMA-idle window begins)
                whb = load_w("whb", Whb, F32R)
                whx = load_w("whx", Whx, F32R)
                whe = load_w("whe", Whe, BF16)
                wm1 = load_w("wm1", Wm1, F32R)
                wm2 = load_w("wm2", Wm2, F32R)
                whm1 = load_w("whm1", Whm1, F32R)
                whm2 = load_w("whm2", Whm2, F32R)

            # ---------------- tail: channel softmax + fold, spatial chain ----------------
            with tc.tile_pool(name="sbT", bufs=1) as sbT:
                psL1_cm = tc.tile_pool(name="psL1", bufs=1, space="PSUM")
                psL = psL1_cm.__enter__()
                # channel softmax over C + Wst2T + fold (PE/DVE/Act while col assembles)
                sm4 = sbT.tile([K16, 4], F32, tag="sm4")
                nc.vector.tensor_copy(s_pre, sps)
                nc.vector.reduce_max(sm4[:, 0:1], s_pre, axis=AX.X)
                nc.vector.tensor_scalar_mul(sm4[:, 1:2], sm4[:, 0:1], -1.0)
                ex = sbT.tile([K16, C], F32, tag="ex")
                nc.scalar.activation(ex, s_pre, AF.Exp, bias=sm4[:, 1:2], scale=1.0)
                nc.vector.reduce_sum(sm4[:, 2:3], ex, axis=AX.X)
                nc.vector.reciprocal(sm4[:, 3:4], sm4[:, 2:3])
                nc.vector.tensor_scalar_mul(S_sb, ex, sm4[:, 3:4])
                for mt in range(2):
                    psT2 = psL.tile([128, K16], F32R, tag="sT")
                    nc.tensor.transpose(psT2, S_sb[:, mt * 128:(mt + 1) * 128],
                                        idnr[0:K16, 0:K16])
                    nc.vector.tensor_copy(sT_sb[:, mt, :], psT2)
                pw = psL.tile([K16, C], F32, tag="pw")
                nc.tensor.matmul(pw, sT_sb[:, 0, :], whb[0], start=True, stop=False)
                nc.tensor.matmul(pw, sT_sb[:, 1, :], whb[1], start=False, stop=True)
                nc.vector.tensor_copy(Wst2T, pw)
                pw2 = psL.tile([128, 2, C], F32, tag="pw2")
                for kt in range(2):
                    nc.tensor.matmul(pw2[:, kt, :], wf16[:, kt * 128:(kt + 1) * 128],
                                     Wst2T, start=True, stop=True, skip_group_check=True)
                for kt in range(2):
                    nc.vector.tensor_tensor(whx_eff[kt], pw2[:, kt, :],
                                            whx[kt].bitcast(F32), op=ALU.add)
                psL1_cm.__exit__(None, None, None)
                psL2_cm = tc.tile_pool(name="psL2", bufs=1, space="PSUM")
                psL = psL2_cm.__enter__()
                psCv_cm = tc.tile_pool(name="psCv", bufs=2, space="PSUM")
                psCv = psCv_cm.__enter__()

                # col matrix: 14 shifted rows (avg 0-6, max 7-13), then the conv
                col = sbT.tile([14, HW], BF16, tag="col")
                nc.gpsimd.memset(col, 0.0)
                for r in range(14):
                    srow = 0 if r < 7 else 2
                    dy = (r - 3) if r < 7 else (r - 10)
                    s0, s1 = max(0, -dy * 96), HW - max(0, dy * 96)
                    nc.sync.dma_start(col[r:r + 1, s0:s1],
                                      avbm[srow:srow + 1, s0 + dy * 96:s1 + dy * 96])
                nc.sync.dma_start(out=bm_pl,
                                  in_=avbm[1:2, :].rearrange("q (h w) -> q h w", w=96))
                c16 = sbT.tile([16, HW], BF16, tag="c16")
                for t in range(18):
                    a = t * 512
                    psc = psCv.tile([16, 512], F32, tag="cps")
                    nc.tensor.matmul(psc, wcol, col[:, a:a + 512], start=True, stop=True)
                    if t % 2 == 0:
                        nc.vector.tensor_copy(c16[:, a:a + 512], psc)
                    else:
                        nc.scalar.copy(c16[:, a:a + 512], psc)

                # planes of c16 rows; conv dx-shifts; F4
                c16_pl = sbT.tile([96, 16, 96], BF16, tag="c16_pl")
                for r in range(16):
                    nc.sync.dma_start(out=c16_pl[:, r, :],
                                      in_=c16[r:r + 1, :].rearrange("q (h w) -> q h w", w=96))
                accs = sbT.tile([96, 4, 96], F32, tag="accs")
                nc.vector.memset(accs, 0.0)
                f4_pl = sbT.tile([96, 4, 96], BF16, tag="f4_pl")
                colmap = [[0], [-1, 0, 1], [-2, -1, 0, 1, 2], [-3, -2, -1, 0, 1, 2, 3]]
                rr = 0
                for kq, dxs in enumerate(colmap):
                    acc = accs[:, kq, :]
                    first = True
                    for dx in dxs:
                        a0, a1 = max(0, -dx), 96 - max(0, dx)
                        srcp = c16_pl[:, rr, a0 + dx:a1 + dx]
                        if first:
                            nc.vector.tensor_copy(acc[:, a0:a1], srcp)
                            first = False
                        else:
                            nc.vector.tensor_tensor(acc[:, a0:a1], acc[:, a0:a1], srcp, op=ALU.add)
                        rr += 1
                    lrelu(f4_pl[:, kq, :], acc)
                for kq in range(4):
                    nc.sync.dma_start(out=f4r[kq:kq + 1, :].rearrange("q (h w) -> q h w", w=96),
                                      in_=f4_pl[:, kq, :])

                # Cm plane; Bm lrelu; spatial logits + softmax
                cmtmp = sbT.tile([96, 96], F32, tag="cmtmp")
                nc.vector.tensor_scalar_mul(cmtmp, f4_pl[:, 0, :], wf2cb[:, 0:1])
                for jj in range(1, 4):
                    nc.vector.scalar_tensor_tensor(cmtmp, f4_pl[:, jj, :], wf2cb[:, jj:jj + 1],
                                                   cmtmp, op0=ALU.mult, op1=ALU.add)
                cm_l = sbT.tile([96, 96], F32R, tag="cm_l")
                lrelu(cm_l, cmtmp)
                bm_l = sbT.tile([96, 96], F32R, tag="bm_l")
                lrelu(bm_l, bm_pl)
                bmT_ps = psL.tile([96, 96], F32R, tag="bmT")
                nc.tensor.transpose(bmT_ps, bm_l, idnr[0:96, 0:96])
                bmT = sbT.tile([96, 96], F32R, tag="bmT")
                nc.vector.tensor_copy(bmT, bmT_ps)
                l_ps = psL.tile([96, 96], F32, tag="lps")
                nc.tensor.matmul(l_ps, bmT, cm_l, start=True, stop=True)
                sv4 = sbT.tile([96, 4], F32, tag="sv4")
                ssp = sbT.tile([96, 96], F32R, tag="ssp")
                nc.vector.reduce_max(sv4[:, 0:1], l_ps, axis=AX.X)
                nc.vector.tensor_scalar_mul(sv4[:, 1:2], sv4[:, 0:1], -1.0)
                nc.scalar.activation(ssp, l_ps, AF.Exp, bias=sv4[:, 1:2], scale=1.0)
                nc.vector.reduce_sum(sv4[:, 2:3], ssp.bitcast(F32), axis=AX.X)
                nc.vector.reciprocal(sv4[:, 3:4], sv4[:, 2:3])
                nc.vector.tensor_scalar_mul(ssp, ssp.bitcast(F32), sv4[:, 3:4])
                sspT_ps = psL.tile([96, 96], F32R, tag="sspT")
                nc.tensor.transpose(sspT_ps, ssp, idnr[0:96, 0:96])
                nc.scalar.copy(sspT_bf, sspT_ps.bitcast(F32))
                psCv_cm.__exit__(None, None, None)
                psL2_cm.__exit__(None, None, None)

        # ---------------- Esp production (software-pipelined) ----------------
        with tc.tile_pool(name="sbE", bufs=1) as sbE, \
             tc.tile_pool(name="psDT", bufs=2, space="PSUM") as psDT, \
             tc.tile_pool(name="psE2", bufs=2, space="PSUM") as psE2:

            def issue_pd(wq):
                pd = psDT.tile([96, 4, C], F32, tag="pd", bufs=2)
                for wi in range(4):
                    nc.tensor.matmul(pd[:, wi, :], f4r[:, 4 * wq + wi::96], wf2d,
                                     start=True, stop=True, skip_group_check=True)
                dt = sbE.tile([96, 4, C], BF16, tag="dt", bufs=4)
                lrelu(dt, pd)
                return dt

            dts = {0: issue_pd(0), 1: issue_pd(1)}
            for wq in range(24):
                if wq + 2 < 24:
                    dts[wq + 2] = issue_pd(wq + 2)
                dt = dts.pop(wq)
                for ch in range(2):
                    pe = psE2.tile([128, 4, 96], F32, tag=f"pe{ch}", name=f"pe{ch}", bufs=2)
                    for wi in range(4):
                        nc.tensor.matmul(pe[:, wi, :], dt[:, wi, ch * 128:(ch + 1) * 128],
                                         sspT_bf, start=True, stop=True, skip_group_check=True)
                    nc.gpsimd.tensor_copy(espT[ch][:, wq * 384:(wq + 1) * 384], pe)

        # ---------------- fusion (streamed 480-px chunks) ----------------
        with tc.tile_pool(name="psF", bufs=8, space="PSUM") as psF, \
             tc.tile_pool(name="sbF", bufs=2) as sbF:
            esp_v = [espT[ch].rearrange("p (w h) -> p h w", h=96) for ch in range(2)]
            for (n0, n) in FCH:
                h0, hn = n0 // 96, n // 96
                hh_c = [sbF.tile([128, 480], BF16, tag=f"h{m}", name=f"h{m}", bufs=2)
                        for m in range(2)]
                m_c = [sbF.tile([128, 480], BF16, tag=f"m{m}", name=f"m{m}", bufs=2)
                       for m in range(2)]
                for mt in range(2):
                    ms = slice(mt * 128, (mt + 1) * 128)
                    ps = psF.tile([128, 480], F32, tag="ps", name=f"psH{mt}")
                    for kt2 in range(2):
                        nc.tensor.matmul(ps[:, 0:n], whe[kt2][:, ms],
                                         esp_v[kt2][:, h0:h0 + hn, :],
                                         start=(kt2 == 0), stop=False)
                    nc.tensor.matmul(ps[:, 0:n], whx_eff[0][:, ms], x0[:, n0:n0 + n],
                                     start=False, stop=False)
                    nc.tensor.matmul(ps[:, 0:n], whx_eff[1][:, ms], x1[:, n0:n0 + n],
                                     start=False, stop=True)
                    lrelu(hh_c[mt][:, 0:n], ps[:, 0:n])
                for mt in range(2):
                    ms = slice(mt * 128, (mt + 1) * 128)
                    ps = psF.tile([128, 480], F32, tag="ps", name=f"psM{mt}")
                    nc.tensor.matmul(ps[:, 0:n], wm1[0][:, ms], hh_c[0][:, 0:n], start=True, stop=False)
                    nc.tensor.matmul(ps[:, 0:n], wm1[1][:, ms], hh_c[1][:, 0:n], start=False, stop=False)
                    nc.tensor.matmul(ps[:, 0:n], wm2[0][:, ms], x0[:, n0:n0 + n], start=False, stop=False)
                    nc.tensor.matmul(ps[:, 0:n], wm2[1][:, ms], x1[:, n0:n0 + n], start=False, stop=True)
                    nc.scalar.activation(m_c[mt][:, 0:n], ps[:, 0:n], AF.Sigmoid)
                for mt in range(2):
                    ms = slice(mt * 128, (mt + 1) * 128)
                    ps = psF.tile([128, 480], F32, tag="ps", name=f"psO{mt}")
                    nc.tensor.matmul(ps[:, 0:n], whm1[0][:, ms], hh_c[0][:, 0:n], start=True, stop=False)
                    nc.tensor.matmul(ps[:, 0:n], whm1[1][:, ms], hh_c[1][:, 0:n], start=False, stop=False)
                    nc.tensor.matmul(ps[:, 0:n], whm2[0][:, ms], m_c[0][:, 0:n], start=False, stop=False)
                    nc.tensor.matmul(ps[:, 0:n], whm2[1][:, ms], m_c[1][:, 0:n], start=False, stop=True)
                    oc = sbF.tile([128, 480], F32, tag="oc")
                    lrelu(oc[:, 0:n], ps[:, 0:n])
                    nc.sync.dma_start(Y[mt * 128:(mt + 1) * 128, n0:n0 + n], oc[:, 0:n])

    if not nc.is_finalized():
        nc.finalize()
    return nc


def _host_weights(w_f, w_beta, w1, w3, w5, w7, w_a2b, w_f2c, w_f2d, w_e, w_h, w_m, w_hm):
    bf = ml_dtypes.bfloat16
    wst = np.concatenate([w_f.T, w_a2b.T, np.full((C, 1), 1.0 / C, np.float32)], axis=1)

    def kt(mat):
        return _round_f32r(np.ascontiguousarray(mat.reshape(2, 128, -1)))

    w_h1, w_h2 = w_h[:, :C], w_h[:, C:]
    wcol = np.zeros((14, 16), np.float32)
    colbase = [0, 1, 4, 9]
    for ki, wk in enumerate([w1, w3, w5, w7]):
        p = (wk.shape[2] - 1) // 2
        for ci in range(2):
            for dy in range(-p, p + 1):
                for dx in range(-p, p + 1):
                    wcol[ci * 7 + dy + 3, colbase[ki] + dx + p] = wk[0, ci, dy + p, dx + p]
    return dict(
        wstr=_round_f32r(wst), wf16=_round_f32r(w_f),
        idnb=np.eye(128, dtype=np.float32).astype(bf),
        idnr=np.eye(128, dtype=np.float32),
        whb=kt((w_h1 @ w_beta).T), whx=kt((w_h1 + w_h2).T),
        wm1=kt(w_m[:, :C].T), wm2=kt(w_m[:, C:].T),
        whm1=kt(w_hm[:, :C].T), whm2=kt(w_hm[:, C:].T),
        whe=np.ascontiguousarray((w_h2 @ w_e).T.reshape(2, 128, C)).astype(bf),
        wf2d=w_f2d.T.astype(bf), wcol=wcol.astype(bf),
        wf2cb=np.ascontiguousarray(w_f2c.astype(np.float32)),
    )


_NC_CACHE = {}


def kernel(x, w_f, w_beta, w1, w3, w5, w7, w_a2b, w_f2c, w_f2d, w_e, w_h, w_m, w_hm,
           _trace=False):
    if "nc" not in _NC_CACHE:
        _NC_CACHE["nc"] = _build_program()
    nc = _NC_CACHE["nc"]

    args = [np.asarray(a, np.float32) for a in
            (w_f, w_beta, w1, w3, w5, w7, w_a2b, w_f2c, w_f2d, w_e, w_h, w_m, w_hm)]
    wts = _host_weights(*args)
    xr = _round_f32r(np.asarray(x, np.float32).reshape(B, C, HW))
    in_maps = [dict(wts, x=np.ascontiguousarray(xr[i])) for i in range(B)]

    kw = dict(trace=True, trace_cores=[0]) if _trace else {}
    r = run_bass_kernel_spmd(nc, in_maps, list(range(B)), **kw)
    out = np.stack([r.results[i]["y"].reshape(C, H, W) for i in range(B)])
    if _trace:
        kernel._last = r
    return out


# revision 24
# speedup vs baseline: 1.0122x; 1.0122x over previous
"""Trainium2 Bass kernel for nn_CPA_CCA_block (channel attention + spatial attention + fusion).

Batch-sharded: 8 samples over 8 NeuronCores, replicated weights, zero collectives.

Schedule (single pass, engines overlapped):
  - x streamed in 512-px chunks; per chunk: stats rows (F/avg/bm), channel max
    (Pool all-reduce), PE transposes of x + exact S logits accumulation, and
    the spatial conv for trailing 480-px windows — all pipelined under the DMA.
  - spatial softmax + channel softmax + (w_h1 w_beta S^T w_f)-fold into the
    x-weights right after the load drains.
  - Esp production software-pipelined (PE) with the D-lrelu split Act/DVE and
    PSUM evacuation on gpsimd.
  - fusion streamed in 480-px chunks; Hh/M kept bf16 (1 cycle/row moving).
Cost-model notes: matmul cost = out free size x cycles/row; transposes use a
bf16 identity (1 c/r); f32r moving needs free>=256 for 1 c/r.
"""
import sys
sys.path.insert(0, '/opt/trn_rl_repo')
import numpy as np
from contextlib import ExitStack

import concourse.bacc as bacc
import concourse.tile as tile
from concourse import mybir
from concourse.bass_utils import run_bass_kernel_spmd
from concourse import bass_isa
import ml_dtypes

F32 = mybir.dt.float32
F32R = mybir.dt.float32r
BF16 = mybir.dt.bfloat16
AF = mybir.ActivationFunctionType
ALU = mybir.AluOpType
AX = mybir.AxisListType

B, C, H, W = 8, 256, 96, 96
HW = H * W
K16 = 16
NEG = 0.001
LCH = [(j * 1536, 1536) for j in range(6)]   # load chunks
FCH = [(k * 480, 480) for k in range(19)] + [(9120, 96)]   # conv/fusion chunks


def _round_f32r(a):
    b = np.ascontiguousarray(a, dtype=np.float32).view(np.uint32)
    lsb = (b >> np.uint32(12)) & np.uint32(1)
    r = (b + np.uint32(0x7FF) + lsb) & np.uint32(0xFFFFF000)
    return r.view(np.float32)


def _build_program():
    nc = bacc.Bacc("TRN2", target_bir_lowering=False, debug=False)

    def din(name, shape, dt):
        return nc.dram_tensor(name, shape, dt, kind="ExternalInput").ap()

    X = din("x", [C, HW], F32R)
    Wstr = din("wstr", [C, 18], F32R)
    Wf16 = din("wf16", [K16, C], F32R)
    IDNR = din("idnr", [128, 128], F32R)
    Wcol = din("wcol", [14, 16], BF16)
    Wf2d = din("wf2d", [4, C], BF16)
    WF2CB = din("wf2cb", [1, 4], F32)
    Whb = din("whb", [2, 128, C], F32R)
    Whx = din("whx", [2, 128, C], F32R)
    Wm1 = din("wm1", [2, 128, C], F32R)
    Wm2 = din("wm2", [2, 128, C], F32R)
    Whm1 = din("whm1", [2, 128, C], F32R)
    Whm2 = din("whm2", [2, 128, C], F32R)
    Whe = din("whe", [2, 128, C], BF16)

    Y = nc.dram_tensor("y", [C, HW], F32, kind="ExternalOutput").ap()

    def lrelu(out, src):
        nc.scalar.activation(out, src, AF.Prelu, alpha=NEG)

    with tile.TileContext(nc) as tc, ExitStack() as ctx:
        per = ctx.enter_context(tc.tile_pool(name="per", bufs=1))
        x0 = per.tile([128, HW], F32R, tag="x0")
        x1 = per.tile([128, HW], F32R, tag="x1")
        xs = [x0, x1]
        f4r = per.tile([4, HW], BF16, tag="f4r")
        bm_pl = per.tile([96, 96], BF16, tag="bm_pl")

        idnr = per.tile([128, 128], F32R, tag="idnr")
        wstr = [per.tile([128, 18], F32R, tag=f"wstr{t}", name=f"wstr{t}") for t in range(2)]
        wf16 = per.tile([K16, C], F32R, tag="wf16")
        wcol = per.tile([14, 16], BF16, tag="wcol")
        wf2d = per.tile([4, C], BF16, tag="wf2d")
        wf2cb = per.tile([96, 4], F32, tag="wf2cb")
        sspT_bf = per.tile([96, 96], BF16, tag="sspT")
        S_sb = per.tile([K16, C], F32R, tag="S_sb")
        s_pre = per.tile([K16, C], F32, tag="s_pre")
        sT_sb = per.tile([128, 2, K16], F32R, tag="sT_sb")
        Wst2T = per.tile([K16, C], F32R, tag="Wst2T")
        whx_eff = [per.tile([128, C], F32R, tag=f"whx_eff{kt}", name=f"whx_eff{kt}")
                   for kt in range(2)]

        def load_w(name, ap, dt):
            ts = []
            for kt in range(2):
                t = per.tile([128, C], dt, tag=f"{name}{kt}", name=f"{name}{kt}")
                nc.sync.dma_start(t, ap[kt])
                ts.append(t)
            return ts

        # preamble: small weights needed during the load phase
        for t in range(2):
            nc.sync.dma_start(wstr[t], Wstr[t * 128:(t + 1) * 128, :])
        nc.sync.dma_start(idnr, IDNR)
        nc.gpsimd.dma_start(out=wf2cb, in_=WF2CB.to_broadcast((96, 4)))

        # ---------------- load phase: x chunks + stats rows + S logits ----------------
        with tc.tile_pool(name="pAvbm", bufs=1) as pAvbm, \
             tc.tile_pool(name="psS", bufs=1, space="PSUM") as psS:
            # rows: 0=avg(bf16), 1=bm, 2=channel-max
            avbm = pAvbm.tile([3, HW], BF16, tag="avbm")
            sps = psS.tile([K16, C], F32, tag="sps")

            with tc.tile_pool(name="sbL", bufs=1) as sbL, \
                 tc.tile_pool(name="psA", bufs=2, space="PSUM") as psA, \
                 tc.tile_pool(name="psT", bufs=3, space="PSUM") as psT, \
                 tc.tile_pool(name="psStF", bufs=2, space="PSUM") as psStF:
                for j, (n0, n) in enumerate(LCH):
                    for ct in range(2):
                        nc.sync.dma_start(xs[ct][:, n0:n0 + n],
                                          X[ct * 128:(ct + 1) * 128, n0:n0 + n])
                    st_sb = sbL.tile([18, 1536], F32R, tag="st_sb", bufs=2)
                    for half in range(3):
                        a = half * 512
                        psa = psA.tile([18, 512], F32, tag="ps")
                        nc.tensor.matmul(psa, wstr[0], x0[:, n0 + a:n0 + a + 512],
                                         start=True, stop=False)
                        nc.tensor.matmul(psa, wstr[1], x1[:, n0 + a:n0 + a + 512],
                                         start=False, stop=True)
                        if half % 2 == 0:
                            nc.vector.tensor_copy(st_sb[:, a:a + 512], psa)
                        else:
                            nc.scalar.copy(st_sb[:, a:a + 512], psa)
                    # avg/bm rows -> bf16 (gpsimd cast DMA, one per chunk)
                    nc.gpsimd.dma_start(out=avbm[0:2, n0:n0 + n], in_=st_sb.bitcast(F32)[16:18, 0:n])
                    # channel max (Pool) -> row 2
                    mx = sbL.tile([128, 1536], BF16, tag="mx", bufs=2)
                    nc.vector.tensor_tensor(mx[:, 0:n], x0[:, n0:n0 + n].bitcast(F32),
                                            x1[:, n0:n0 + n].bitcast(F32), op=ALU.max)
                    nc.gpsimd.partition_all_reduce(mx[:, 0:n], mx[:, 0:n], channels=128,
                                                   reduce_op=bass_isa.ReduceOp.max)
                    nc.sync.dma_start(avbm[2:3, n0:n0 + n], mx[0:1, 0:n])
                    # F^T chunks for the exact S logits
                    stf_ps = psStF.tile([128, 12, K16], F32R, tag="stf")
                    for s in range(12):
                        nc.tensor.transpose(stf_ps[:, s, :],
                                            st_sb[0:16, s * 128:(s + 1) * 128],
                                            idnr[0:K16, 0:K16])
                    stf_sb = sbL.tile([128, 12, K16], F32R, tag="stf_sb", bufs=2)
                    nc.vector.tensor_copy(stf_sb, stf_ps)
                    # x^T subchunk pairs + S logit accumulation (full-precision path)
                    for sp in range(6):
                        pt = psT.tile([128, 2, 2, 128], F32R, tag="pt")
                        for si in range(2):
                            m0 = n0 + (2 * sp + si) * 128
                            for ct in range(2):
                                nc.tensor.transpose(pt[:, si, ct, :], xs[ct][:, m0:m0 + 128], idnr)
                        xt = sbL.tile([128, 2, 2, 128], F32R, tag="xt", bufs=4)
                        if sp % 3 == 2:
                            nc.vector.tensor_copy(xt, pt)
                        else:
                            nc.scalar.copy(xt, pt)
                        for si in range(2):
                            s = 2 * sp + si
                            nc.tensor.matmul(sps, stf_sb[:, s, :],
                                             xt[:, si, :, :].rearrange("p a b -> p (a b)"),
                                             start=(j == 0 and s == 0),
                                             stop=(j == len(LCH) - 1 and s == 11))

                # remaining big weights (DMA-idle window begins)
                nc.sync.dma_start(wf16, Wf16)
                nc.sync.dma_start(wcol, Wcol)
                nc.sync.dma_start(wf2d, Wf2d)
                whb = load_w("whb", Whb, F32R)
                whx = load_w("whx", Whx, F32R)
                whe = load_w("whe", Whe, BF16)
                wm1 = load_w("wm1", Wm1, F32R)
                wm2 = load_w("wm2", Wm2, F32R)
                whm1 = load_w("whm1", Whm1, F32R)
                whm2 = load_w("whm2", Whm2, F32R)

            # ---------------- tail: channel softmax + fold, spatial chain ----------------
            with tc.tile_pool(name="sbT", bufs=1) as sbT:
                psL1_cm = tc.tile_pool(name="psL1", bufs=1, space="PSUM")
                psL = psL1_cm.__enter__()
                # channel softmax over C + Wst2T + fold (PE/DVE/Act while col assembles)
                sm4 = sbT.tile([K16, 4], F32, tag="sm4")
                nc.vector.tensor_copy(s_pre, sps)
                nc.vector.reduce_max(sm4[:, 0:1], s_pre, axis=AX.X)
                nc.vector.tensor_scalar_mul(sm4[:, 1:2], sm4[:, 0:1], -1.0)
                ex = sbT.tile([K16, C], F32, tag="ex")
                nc.scalar.activation(ex, s_pre, AF.Exp, bias=sm4[:, 1:2], scale=1.0)
                nc.vector.reduce_sum(sm4[:, 2:3], ex, axis=AX.X)
                nc.vector.reciprocal(sm4[:, 3:4], sm4[:, 2:3])
                nc.vector.tensor_scalar_mul(S_sb, ex, sm4[:, 3:4])
                for mt in range(2):
                    psT2 = psL.tile([128, K16], F32R, tag="sT")
                    nc.tensor.transpose(psT2, S_sb[:, mt * 128:(mt + 1) * 128],
                                        idnr[0:K16, 0:K16])
                    nc.vector.tensor_copy(sT_sb[:, mt, :], psT2)
                pw = psL.tile([K16, C], F32, tag="pw")
                nc.tensor.matmul(pw, sT_sb[:, 0, :], whb[0], start=True, stop=False)
                nc.tensor.matmul(pw, sT_sb[:, 1, :], whb[1], start=False, stop=True)
                nc.vector.tensor_copy(Wst2T, pw)
                pw2 = psL.tile([128, 2, C], F32, tag="pw2")
                for kt in range(2):
                    nc.tensor.matmul(pw2[:, kt, :], wf16[:, kt * 128:(kt + 1) * 128],
                                     Wst2T, start=True, stop=True, skip_group_check=True)
                for kt in range(2):
                    nc.vector.tensor_tensor(whx_eff[kt], pw2[:, kt, :],
                                            whx[kt].bitcast(F32), op=ALU.add)
                psL1_cm.__exit__(None, None, None)
                psL2_cm = tc.tile_pool(name="psL2", bufs=1, space="PSUM")
                psL = psL2_cm.__enter__()
                psCv_cm = tc.tile_pool(name="psCv", bufs=2, space="PSUM")
                psCv = psCv_cm.__enter__()

                # col matrix: 14 shifted rows (avg 0-6, max 7-13), then the conv
                col = sbT.tile([14, HW], BF16, tag="col")
                nc.gpsimd.memset(col, 0.0)
                for r in range(14):
                    srow = 0 if r < 7 else 2
                    dy = (r - 3) if r < 7 else (r - 10)
                    s0, s1 = max(0, -dy * 96), HW - max(0, dy * 96)
                    nc.sync.dma_start(col[r:r + 1, s0:s1],
                                      avbm[srow:srow + 1, s0 + dy * 96:s1 + dy * 96])
                nc.sync.dma_start(out=bm_pl,
                                  in_=avbm[1:2, :].rearrange("q (h w) -> q h w", w=96))
                c16 = sbT.tile([16, HW], BF16, tag="c16")
                for t in range(18):
                    a = t * 512
                    psc = psCv.tile([16, 512], F32, tag="cps")
                    nc.tensor.matmul(psc, wcol, col[:, a:a + 512], start=True, stop=True)
                    if t % 2 == 0:
                        nc.vector.tensor_copy(c16[:, a:a + 512], psc)
                    else:
                        nc.scalar.copy(c16[:, a:a + 512], psc)

                # planes of c16 rows; conv dx-shifts; F4
                c16_pl = sbT.tile([96, 16, 96], BF16, tag="c16_pl")
                for r in range(16):
                    nc.sync.dma_start(out=c16_pl[:, r, :],
                                      in_=c16[r:r + 1, :].rearrange("q (h w) -> q h w", w=96))
                accs = sbT.tile([96, 4, 96], F32, tag="accs")
                nc.vector.memset(accs, 0.0)
                f4_pl = sbT.tile([96, 4, 96], BF16, tag="f4_pl")
                colmap = [[0], [-1, 0, 1], [-2, -1, 0, 1, 2], [-3, -2, -1, 0, 1, 2, 3]]
                rr = 0
                for kq, dxs in enumerate(colmap):
                    acc = accs[:, kq, :]
                    first = True
                    for dx in dxs:
                        a0, a1 = max(0, -dx), 96 - max(0, dx)
                        srcp = c16_pl[:, rr, a0 + dx:a1 + dx]
                        if first:
                            nc.vector.tensor_copy(acc[:, a0:a1], srcp)
                            first = False
                        else:
                            nc.vector.tensor_tensor(acc[:, a0:a1], acc[:, a0:a1], srcp, op=ALU.add)
                        rr += 1
                    lrelu(f4_pl[:, kq, :], acc)
                for kq in range(4):
                    nc.sync.dma_start(out=f4r[kq:kq + 1, :].rearrange("q (h w) -> q h w", w=96),
                                      in_=f4_pl[:, kq, :])

                # Cm plane; Bm lrelu; spatial logits + softmax
                cmtmp = sbT.tile([96, 96], F32, tag="cmtmp")
                nc.vector.tensor_scalar_mul(cmtmp, f4_pl[:, 0, :], wf2cb[:, 0:1])
                for jj in range(1, 4):
                    nc.vector.scalar_tensor_tensor(cmtmp, f4_pl[:, jj, :], wf2cb[:, jj:jj + 1],
                                                   cmtmp, op0=ALU.mult, op1=ALU.add)
                cm_l = sbT.tile([96, 96], F32R, tag="cm_l")
                lrelu(cm_l, cmtmp)
                bm_l = sbT.tile([96, 96], F32R, tag="bm_l")
                lrelu(bm_l, bm_pl)
                bmT_ps = psL.tile([96, 96], F32R, tag="bmT")
                nc.tensor.transpose(bmT_ps, bm_l, idnr[0:96, 0:96])
                bmT = sbT.tile([96, 96], F32R, tag="bmT")
                nc.vector.tensor_copy(bmT, bmT_ps)
                l_ps = psL.tile([96, 96], F32, tag="lps")
                nc.tensor.matmul(l_ps, bmT, cm_l, start=True, stop=True)
                sv4 = sbT.tile([96, 4], F32, tag="sv4")
                ssp = sbT.tile([96, 96], F32R, tag="ssp")
                nc.vector.reduce_max(sv4[:, 0:1], l_ps, axis=AX.X)
                nc.vector.tensor_scalar_mul(sv4[:, 1:2], sv4[:, 0:1], -1.0)
                nc.scalar.activation(ssp, l_ps, AF.Exp, bias=sv4[:, 1:2], scale=1.0)
                nc.vector.reduce_sum(sv4[:, 2:3], ssp.bitcast(F32), axis=AX.X)
                nc.vector.reciprocal(sv4[:, 3:4], sv4[:, 2:3])
                nc.vector.tensor_scalar_mul(ssp, ssp.bitcast(F32), sv4[:, 3:4])
                sspT_ps = psL.tile([96, 96], F32R, tag="sspT")
                nc.tensor.transpose(sspT_ps, ssp, idnr[0:96, 0:96])
                nc.scalar.copy(sspT_bf, sspT_ps.bitcast(F32))
                psCv_cm.__exit__(None, None, None)
                psL2_cm.__exit__(None, None, None)

        # ---------------- Esp production (software-pipelined) ----------------
        with tc.tile_pool(name="sbE", bufs=1) as sbE, \
             tc.tile_pool(name="psDT", bufs=2, space="PSUM") as psDT, \
             tc.tile_pool(name="psE2", bufs=2, space="PSUM") as psE2:

            def issue_pd(wq):
                pd = psDT.tile([96, 4, C], F32, tag="pd", bufs=2)
                for wi in range(4):
                    nc.tensor.matmul(pd[:, wi, :], f4r[:, 4 * wq + wi::96], wf2d,
                                     start=True, stop=True, skip_group_check=True)
                dt = sbE.tile([96, 4, C], BF16, tag="dt", bufs=4)
                lrelu(dt, pd)
                return dt

            dts = {0: issue_pd(0), 1: issue_pd(1)}
            for wq in range(24):
                if wq + 2 < 24:
                    dts[wq + 2] = issue_pd(wq + 2)
                dt = dts.pop(wq)
                for ch in range(2):
                    pe = psE2.tile([128, 4, 96], F32, tag=f"pe{ch}", name=f"pe{ch}", bufs=2)
                    for wi in range(4):
                        nc.tensor.matmul(pe[:, wi, :], dt[:, wi, ch * 128:(ch + 1) * 128],
                                         sspT_bf, start=True, stop=True, skip_group_check=True)
                    nc.gpsimd.tensor_copy(espT[ch][:, wq * 384:(wq + 1) * 384], pe)

        # ---------------- fusion (streamed 480-px chunks) ----------------
        with tc.tile_pool(name="psF", bufs=8, space="PSUM") as psF, \
             tc.tile_pool(name="sbF", bufs=2) as sbF:
            esp_v = [espT[ch].rearrange("p (w h) -> p h w", h=96) for ch in range(2)]
            for (n0, n) in FCH:
                h0, hn = n0 // 96, n // 96
                hh_c = [sbF.tile([128, 480], BF16, tag=f"h{m}", name=f"h{m}", bufs=2)
                        for m in range(2)]
                m_c = [sbF.tile([128, 480], BF16, tag=f"m{m}", name=f"m{m}", bufs=2)
                       for m in range(2)]
                for mt in range(2):
                    ms = slice(mt * 128, (mt + 1) * 128)
                    ps = psF.tile([128, 480], F32, tag="ps", name=f"psH{mt}")
                    for kt2 in range(2):
                        nc.tensor.matmul(ps[:, 0:n], whe[kt2][:, ms],
                                         esp_v[kt2][:, h0:h0 + hn, :],
                                         start=(kt2 == 0), stop=False)
                    nc.tensor.matmul(ps[:, 0:n], whx_eff[0][:, ms], x0[:, n0:n0 + n],
                                     start=False, stop=False)
                    nc.tensor.matmul(ps[:, 0:n], whx_eff[1][:, ms], x1[:, n0:n0 + n],
                                     start=False, stop=True)
                    lrelu(hh_c[mt][:, 0:n], ps[:, 0:n])
                for mt in range(2):
                    ms = slice(mt * 128, (mt + 1) * 128)
                    ps = psF.tile([128, 480], F32, tag="ps", name=f"psM{mt}")
                    nc.tensor.matmul(ps[:, 0:n], wm1[0][:, ms], hh_c[0][:, 0:n], start=True, stop=False)
                    nc.tensor.matmul(ps[:, 0:n], wm1[1][:, ms], hh_c[1][:, 0:n], start=False, stop=False)
                    nc.tensor.matmul(ps[:, 0:n], wm2[0][:, ms], x0[:, n0:n0 + n], start=False, stop=False)
                    nc.tensor.matmul(ps[:, 0:n], wm2[1][:, ms], x1[:, n0:n0 + n], start=False, stop=True)
                    nc.scalar.activation(m_c[mt][:, 0:n], ps[:, 0:n], AF.Sigmoid)
                for mt in range(2):
                    ms = slice(mt * 128, (mt + 1) * 128)
                    ps = psF.tile([128, 480], F32, tag="ps", name=f"psO{mt}")
                    nc.tensor.matmul(ps[:, 0:n], whm1[0][:, ms], hh_c[0][:, 0:n], start=True, stop=False)
                    nc.tensor.matmul(ps[:, 0:n], whm1[1][:, ms], hh_c[1][:, 0:n], start=False, stop=False)
                    nc.tensor.matmul(ps[:, 0:n], whm2[0][:, ms], m_c[0][:, 0:n], start=False, stop=False)
                    nc.tensor.matmul(ps[:, 0:n], whm2[1][:, ms], m_c[1][:, 0:n], start=False, stop=True)
                    oc = sbF.tile([128, 480], F32, tag="oc")
                    lrelu(oc[:, 0:n], ps[:, 0:n])
                    nc.sync.dma_start(Y[mt * 128:(mt + 1) * 128, n0:n0 + n], oc[:, 0:n])

    if not nc.is_finalized():
        nc.finalize()
    return nc


def _host_weights(w_f, w_beta, w1, w3, w5, w7, w_a2b, w_f2c, w_f2d, w_e, w_h, w_m, w_hm):
    bf = ml_dtypes.bfloat16
    wst = np.concatenate([w_f.T, w_a2b.T, np.full((C, 1), 1.0 / C, np.float32)], axis=1)

    def kt(mat):
        return _round_f32r(np.ascontiguousarray(mat.reshape(2, 128, -1)))

    w_h1, w_h2 = w_h[:, :C], w_h[:, C:]
    wcol = np.zeros((14, 16), np.float32)
    colbase = [0, 1, 4, 9]
    for ki, wk in enumerate([w1, w3, w5, w7]):
        p = (wk.shape[2] - 1) // 2
        for ci in range(2):
            for dy in range(-p, p + 1):
                for dx in range(-p, p + 1):
                    wcol[ci * 7 + dy + 3, colbase[ki] + dx + p] = wk[0, ci, dy + p, dx + p]
    return dict(
        wstr=_round_f32r(wst), wf16=_round_f32r(w_f),
        idnr=np.eye(128, dtype=np.float32),
        whb=kt((w_h1 @ w_beta).T), whx=kt((w_h1 + w_h2).T),
        wm1=kt(w_m[:, :C].T), wm2=kt(w_m[:, C:].T),
        whm1=kt(w_hm[:, :C].T), whm2=kt(w_hm[:, C:].T),
        whe=np.ascontiguousarray((w_h2 @ w_e).T.reshape(2, 128, C)).astype(bf),
        wf2d=w_f2d.T.astype(bf), wcol=wcol.astype(bf),
        wf2cb=np.ascontiguousarray(w_f2c.astype(np.float32)),
    )


_NC_CACHE = {}


def kernel(x, w_f, w_beta, w1, w3, w5, w7, w_a2b, w_f2c, w_f2d, w_e, w_h, w_m, w_hm,
           _trace=False):
    if "nc" not in _NC_CACHE:
        _NC_CACHE["nc"] = _build_program()
    nc = _NC_CACHE["nc"]

    args = [np.asarray(a, np.float32) for a in
            (w_f, w_beta, w1, w3, w5, w7, w_a2b, w_f2c, w_f2d, w_e, w_h, w_m, w_hm)]
    wts = _host_weights(*args)
    xr = _round_f32r(np.asarray(x, np.float32).reshape(B, C, HW))
    in_maps = [dict(wts, x=np.ascontiguousarray(xr[i])) for i in range(B)]

    kw = dict(trace=True, trace_cores=[0]) if _trace else {}
    r = run_bass_kernel_spmd(nc, in_maps, list(range(B)), **kw)
    out = np.stack([r.results[i]["y"].reshape(C, H, W) for i in range(B)])
    if _trace:
        kernel._last = r
    return out
